# revision 1
# baseline (speedup 1.0000x reference)
"""Trainium2 Bass kernel for per-sample reflect-pad + random-crop +
brightness/contrast jitter + quantize (DRAC transform).

Contract: kernel(**inputs) takes the FULL unsharded inputs
(x_uint8 [2048,3,64,64] int32, offs_h/offs_w [2048] int32,
jitter_b/jitter_c [2048,1,1,1] float32) and returns the FULL
[2048,3,64,64] int32 output.

Strategy (pure data parallel, batch sharded over 8 cores; all image
compute on device, host does only layout prep):
- Host per core: reflect-pad to [256,3,70,70] uint8 (lossless repack of
  the 0..255-valued int32 input), flatten; turn (offs_h, offs_w) into
  one int32 element offset per (sample, channel) pair; replicate the
  per-sample jitter scalars per pair.
- Device (one SPMD Bass program on 8 cores), 6 chunks x 128 pairs:
  * indirect DMA (gpsimd SWDGE): for each of the 128 destination
    partitions, stream 4480 contiguous uint8 elements starting at
    base + oh*70 + ow. The crop window is then the static strided view
    [128][64 rows, stride 70][64 cols] of that slab - identical for all
    partitions, read directly by the compute engines.
  * spatial sum per pair: one accumulate pass (DVE tensor_scalar with
    accum_out, or ACT Identity with accum_out - engine chosen per chunk
    to balance load). fp32 accumulation of integers <= 255*4096 < 2^24
    is exact.
  * tiny per-pair DVE chain: f = 0.1*jc+0.95, d = 25.5*jb-12.75,
    b = sum*(1-f)/4096 + d
  * fused epilogue, one op: z = x*f + b with uint8 output; the
    narrowing convert saturates to [0,255] and rounds half-to-even,
    which equals round(clip(z,0,255)) == the reference's
    round(clip(.)*255) in the 255-scaled space.
  * store uint8; host casts back to int32 (lossless).
"""
import numpy as np

PAD = 3
B, C, H, W = 2048, 3, 64, 64
HP, WP = H + 2 * PAD, W + 2 * PAD          # 70, 70
NCORES = 8
BS = B // NCORES                            # 256 samples per core
NPAIR = BS * C                              # 768 (sample, channel) pairs
PX = H * W                                  # 4096
SROW = C * HP * WP                          # 14700 elements per padded sample
CHP = 128                                   # pairs per chunk
NCHUNK = NPAIR // CHP                       # 6
SLAB = (H - 1) * WP + WP                    # 4480 contiguous elems per pair

_prog = None                                # compiled Bass program (built once)

# schedule knobs
XBUFS, SBUFS, OBUFS, TBUFS, DBUFS = NCHUNK, 2, NCHUNK, 8, 3
SUM_ON_ACT = (1, 3)
Z_ON_ACT = (2, 4)
SKEW = 2
OFFS_FROM_DRAM = False                      # HW requires indirect offsets in SBUF
SPLIT_FIRST_CROP = True                     # halve chunk-0 crop DMA + partial sums
SPLIT_LAST_Z = True                         # halve last chunk's z + store
HALF = SLAB // 2                            # 2240 = 32 rows of 70


def _build_program():
    from contextlib import ExitStack
    from concourse import bass, bacc, mybir, tile

    f32, i32, u8 = mybir.dt.float32, mybir.dt.int32, mybir.dt.uint8
    AF = mybir.ActivationFunctionType
    OP = mybir.AluOpType

    nc = bacc.Bacc("TRN2", target_bir_lowering=False, debug=False)
    xp = nc.dram_tensor("xp", [1, BS * SROW + SLAB], u8, kind="ExternalInput")
    # columns NCHUNK..2*NCHUNK-1 = offsets shifted by HALF (each chunk's crop
    # is issued as two half-slab DMAs to engage two DMA queues)
    idx = nc.dram_tensor("idx", [CHP, 2 * NCHUNK], i32, kind="ExternalInput")
    jbr = nc.dram_tensor("jbr", [CHP, NCHUNK], f32, kind="ExternalInput")
    jcr = nc.dram_tensor("jcr", [CHP, NCHUNK], f32, kind="ExternalInput")
    out = nc.dram_tensor("out", [NPAIR, PX], u8, kind="ExternalOutput")

    with tile.TileContext(nc) as tc, ExitStack() as ctx:
        const = ctx.enter_context(tc.tile_pool(name="const", bufs=1))
        # dep-free dummy activation at t~0 so the compile pass hoists the
        # ACT function-table load to the very start instead of blocking the
        # first real activation
        warm = const.tile([1, 1], f32)
        nc.gpsimd.memset(warm[:], 0.0)
        nc.scalar.activation(warm[:], warm[:], AF.Identity)
        if not OFFS_FROM_DRAM:
            idx_t = const.tile([CHP, 2 * NCHUNK], i32)
            nc.sync.dma_start(idx_t[:], idx[:, :])
        jb_t = const.tile([CHP, NCHUNK], f32)
        nc.sync.dma_start(jb_t[:], jbr[:, :])
        jc_t = const.tile([CHP, NCHUNK], f32)
        nc.sync.dma_start(jc_t[:], jcr[:, :])

        xpool = ctx.enter_context(tc.tile_pool(name="x", bufs=XBUFS))
        spool = ctx.enter_context(tc.tile_pool(name="s", bufs=SBUFS))
        opool = ctx.enter_context(tc.tile_pool(name="o", bufs=OBUFS))
        tpool = ctx.enter_context(tc.tile_pool(name="t", bufs=TBUFS))
        dpool = ctx.enter_context(tc.tile_pool(name="dump", bufs=DBUFS))

        def emit_head(ci, xv):
            """spatial sum of chunk ci"""
            isum = tpool.tile([CHP, 1], f32, tag="isumf")
            if ci in SUM_ON_ACT:
                scratch = spool.tile([CHP, PX], f32, tag="scr")
                scr3 = scratch[:].rearrange("p (h w) -> p h w", h=H, w=W)
                nc.scalar.activation(scr3, xv, AF.Identity,
                                     bias=0.0, scale=1.0, accum_out=isum[:])
            elif SPLIT_FIRST_CROP and ci == 0:
                # two half-sums so the first starts as soon as half-slab A lands
                ia = tpool.tile([CHP, 1], f32, tag="ia")
                ib = tpool.tile([CHP, 1], f32, tag="ib")
                for hh, acc in ((0, ia), (1, ib)):
                    xh = xv[:, hh * (H // 2):(hh + 1) * (H // 2), :]
                    dp = dpool.tile([CHP, PX // 2], u8, tag="dump")
                    d3 = dp[:].rearrange("p (h w) -> p h w", h=H // 2, w=W)
                    nc.vector.tensor_scalar(d3, xh, 1.0, 0.0, OP.mult, OP.add,
                                            accum_out=acc[:])
                nc.vector.tensor_tensor(isum[:], ia[:], ib[:], OP.add)
            else:
                dump = dpool.tile([CHP, PX], u8, tag="dump")
                d3 = dump[:].rearrange("p (h w) -> p h w", h=H, w=W)
                nc.vector.tensor_scalar(d3, xv, 1.0, 0.0, OP.mult, OP.add,
                                        accum_out=isum[:])
            return isum

        def emit_scalars(ci):
            """f, d, omf4096 = (1-f)/4096 - no dependency on the image sum"""
            fT = tpool.tile([CHP, 1], f32, tag="f")
            nc.vector.tensor_scalar(fT[:], jc_t[:, ci:ci + 1], 0.1, 0.95, OP.mult, OP.add)
            dT = tpool.tile([CHP, 1], f32, tag="d")
            nc.vector.tensor_scalar(dT[:], jb_t[:, ci:ci + 1], 25.5, -12.75, OP.mult, OP.add)
            o4 = tpool.tile([CHP, 1], f32, tag="o4")
            nc.vector.tensor_scalar(o4[:], fT[:], -1.0 / PX, 1.0 / PX, OP.mult, OP.add)
            return fT, dT, o4

        def emit_tail(ci, xv, isum, fT, dT, o4):
            """b = (isum*(1-f)/4096) + d in one DVE op; fused z+convert; store"""
            bT = tpool.tile([CHP, 1], f32, tag="b")
            nc.vector.scalar_tensor_tensor(bT[:], isum[:], o4[:], dT[:],
                                           OP.mult, OP.add)

            zu = opool.tile([CHP, PX], u8, tag="zu")
            z3 = zu[:].rearrange("p (h w) -> p h w", h=H, w=W)
            if SPLIT_LAST_Z and ci == NCHUNK - 1 and ci not in Z_ON_ACT:
                # stream the tail: z + store in halves so the final store
                # overlaps the final compute
                for hh in range(2):
                    sl = slice(hh * (PX // 2), (hh + 1) * (PX // 2))
                    zh = zu[:, sl].rearrange("p (h w) -> p h w", h=H // 2, w=W)
                    xh = xv[:, hh * (H // 2):(hh + 1) * (H // 2), :]
                    nc.vector.tensor_scalar(zh, xh, fT[:], bT[:], OP.mult, OP.add)
                    nc.sync.dma_start(out[CHP * ci:CHP * (ci + 1), sl], zu[:, sl])
                return
            if ci in Z_ON_ACT:
                nc.scalar.activation(z3, xv, AF.Identity, bias=bT[:], scale=fT[:])
            else:
                nc.vector.tensor_scalar(z3, xv, fT[:], bT[:], OP.mult, OP.add)
            # two half stores -> two HWDGE queues in parallel
            nc.sync.dma_start(out[CHP * ci:CHP * (ci + 1), :PX // 2], zu[:, :PX // 2])
            nc.sync.dma_start(out[CHP * ci:CHP * (ci + 1), PX // 2:], zu[:, PX // 2:])

        staged = []
        for ci in range(NCHUNK):
            slab = xpool.tile([CHP, SLAB], u8, tag="slab")
            nc.gpsimd.indirect_dma_start(
                out=slab[:, :HALF], out_offset=None, in_=xp[:, :],
                in_offset=bass.IndirectOffsetOnAxis(ap=idx_t[:, ci:ci + 1], axis=1))
            nc.gpsimd.indirect_dma_start(
                out=slab[:, HALF:], out_offset=None, in_=xp[:, :],
                in_offset=bass.IndirectOffsetOnAxis(
                    ap=idx_t[:, NCHUNK + ci:NCHUNK + ci + 1], axis=1))
            # static strided crop view: [128][64 rows, stride 70][64 cols]
            xv = slab[:, :H * WP].rearrange("p (h w) -> p h w", h=H, w=WP)[:, :, :W]
            scal = emit_scalars(ci)
            isum = emit_head(ci, xv)
            staged.append((ci, xv, isum, *scal))
            if len(staged) > SKEW:
                emit_tail(*staged.pop(0))
        while staged:
            emit_tail(*staged.pop(0))

    nc.compile()
    return nc


def _host_prep(x_uint8, offs_h, offs_w, jitter_b, jitter_c):
    """Shard + build per-core input maps (padding, dtype repack, and index
    arithmetic only - no image math)."""
    xpad = np.pad(np.asarray(x_uint8).astype(np.uint8),
                  ((0, 0), (0, 0), (PAD, PAD), (PAD, PAD)), mode="reflect")
    oh = np.asarray(offs_h).astype(np.int64).reshape(B)
    ow = np.asarray(offs_w).astype(np.int64).reshape(B)
    jb = np.asarray(jitter_b, dtype=np.float32).reshape(B)
    jc = np.asarray(jitter_c, dtype=np.float32).reshape(B)

    c_ar = np.arange(C, dtype=np.int64)
    in_maps = []
    for k in range(NCORES):
        sl = slice(k * BS, (k + 1) * BS)
        ohk, owk = oh[sl], ow[sl]
        base = np.arange(BS, dtype=np.int64) * SROW
        full = (base[:, None] + c_ar[None, :] * (HP * WP)
                + ohk[:, None] * WP + owk[:, None])                 # [BS, C]
        idxm = full.reshape(NPAIR).reshape(NCHUNK, CHP).T.astype(np.int32)
        idx = np.concatenate([idxm, idxm + HALF], axis=1)
        jbr = np.repeat(jb[sl], C).reshape(NCHUNK, CHP).T.copy()
        jcr = np.repeat(jc[sl], C).reshape(NCHUNK, CHP).T.copy()
        xpf = np.zeros((1, BS * SROW + SLAB), np.uint8)
        xpf[0, :BS * SROW] = xpad[sl].reshape(-1)
        in_maps.append({"xp": xpf, "idx": idx, "jbr": jbr, "jcr": jcr})
    return in_maps


# test-harness knobs (ignored by the grading path)
TRACE = False
LAST_RESULT = None


def kernel(x_uint8, offs_h, offs_w, jitter_b, jitter_c):
    global _prog, LAST_RESULT
    from concourse.bass_utils import run_bass_kernel_spmd

    if _prog is None:
        _prog = _build_program()

    in_maps = _host_prep(x_uint8, offs_h, offs_w, jitter_b, jitter_c)
    res = run_bass_kernel_spmd(_prog, in_maps, list(range(NCORES)), trace=TRACE)
    LAST_RESULT = res
    outs = [res.results[k]["out"].reshape(BS, C, H, W) for k in range(NCORES)]
    return np.concatenate(outs, axis=0).astype(np.int32)  # lossless: values in [0,255]



# revision 2
# speedup vs baseline: 1.2761x; 1.2761x over previous
"""Trainium2 Bass kernel for per-sample reflect-pad + random-crop +
brightness/contrast jitter + quantize (DRAC transform).

Design notes (found via TimelineSim + HW micro-benchmarks):
- Gather: ONE indirect-DMA descriptor per sample (a contiguous 14700B
  slab covering all 3 channel crops), 128 per chunk, 256 per core.
  Real SWDGE desc-gen costs ~30ns/descriptor (4x the cost model), so
  descriptor count matters more than fine-grained gather splits; the
  v1 baseline's per-(sample,channel)-half gather used 1536.
- Compute: per (chunk, channel) path config. P1: ACT does the spatial
  sum via activation(Identity, accum_out=...) whose main output doubles
  as a CONTIGUOUS u8 copy of the strided crop, then DVE computes
  z = x*f + b from that copy at full rate (2.2us vs 4.3us strided).
  P3 (one channel per program): DVE strided sum + strided z, which
  offloads the ACT critical chain. Sums/z interleave so both engines
  stay busy; per-channel stores drain as soon as each z finishes.
- Emission order g0,s0,g1,t0,s1,t1 keeps both engine queues free of
  head-of-line blocking on the later chunk's data.

Math (255-scaled space; output convert saturates+rounds to u8):
  f = 0.1*jc + 0.95
  d = 25.5*jb - 12.75
  b_c = sum_c * (1-f)/4096 + d
  z = clip(round(x*f + b_c), 0, 255)
"""
import numpy as np

PAD = 3
B, C, H, W = 2048, 3, 64, 64
HP, WP = H + 2 * PAD, W + 2 * PAD          # 70, 70
NCORES = 8
BS = B // NCORES                            # 256 samples per core
SROW = C * HP * WP                          # 14700 elements per padded sample
CSTR = HP * WP                              # 4900 per channel
PX = H * W                                  # 4096
OUTW = C * PX                               # 12288
CHP = 128                                   # samples per chunk (partition dim)
NCHUNK = BS // CHP                          # 2

# engine path per (chunk, channel): "P1" ACT sum -> DVE z (contiguous),
# "P3" DVE strided sum -> DVE strided z, "P2" DVE strided sum -> ACT z.
PATH = {(0, 0): "P3", (0, 1): "P1", (0, 2): "P1",
        (1, 0): "P1", (1, 1): "P1", (1, 2): "P1"}

_prog = None
TRACE = False
LAST_RESULT = None


def _build_program():
    from contextlib import ExitStack
    from concourse import bass, bacc, mybir, tile

    f32, i32, u8 = mybir.dt.float32, mybir.dt.int32, mybir.dt.uint8
    AF = mybir.ActivationFunctionType
    OP = mybir.AluOpType
    AX = mybir.AxisListType

    nc = bacc.Bacc("TRN2", target_bir_lowering=False, debug=False)
    xp = nc.dram_tensor("xp", [1, BS * SROW + 1024], u8, kind="ExternalInput")
    idx = nc.dram_tensor("idx", [CHP, NCHUNK], i32, kind="ExternalInput")
    jbr = nc.dram_tensor("jbr", [CHP, NCHUNK], f32, kind="ExternalInput")
    jcr = nc.dram_tensor("jcr", [CHP, NCHUNK], f32, kind="ExternalInput")
    out = nc.dram_tensor("out", [BS, OUTW], u8, kind="ExternalOutput")

    with tile.TileContext(nc) as tc, ExitStack() as ctx:
        const = ctx.enter_context(tc.tile_pool(name="const", bufs=1))
        idx_t = const.tile([CHP, NCHUNK], i32)
        nc.sync.dma_start(idx_t[:], idx[:, :])

        # dep-free dummy activation hoists the ACT function-table load
        warm = const.tile([1, 1], f32)
        nc.gpsimd.memset(warm[:], 0.0)
        nc.scalar.activation(warm[:], warm[:], AF.Identity)

        jb_t = const.tile([CHP, NCHUNK], f32)
        nc.sync.dma_start(jb_t[:], jbr[:, :])
        jc_t = const.tile([CHP, NCHUNK], f32)
        nc.sync.dma_start(jc_t[:], jcr[:, :])

        xpool = ctx.enter_context(tc.tile_pool(name="x", bufs=2))
        dpool = ctx.enter_context(tc.tile_pool(name="dmp", bufs=2))
        opool = ctx.enter_context(tc.tile_pool(name="o", bufs=2))
        tpool = ctx.enter_context(tc.tile_pool(name="t", bufs=NCHUNK))

        scal = []
        for ci in range(NCHUNK):
            fT = tpool.tile([CHP, 1], f32, tag="f")
            nc.vector.tensor_scalar(fT[:], jc_t[:, ci:ci + 1], 0.1, 0.95,
                                    OP.mult, OP.add)
            dT = tpool.tile([CHP, 1], f32, tag="d")
            nc.vector.tensor_scalar(dT[:], jb_t[:, ci:ci + 1], 25.5, -12.75,
                                    OP.mult, OP.add)
            o4 = tpool.tile([CHP, 1], f32, tag="o4")
            nc.vector.tensor_scalar(o4[:], fT[:], -1.0 / PX, 1.0 / PX,
                                    OP.mult, OP.add)
            scal.append((fT, dT, o4))

        def crop3d(tl, base):
            v = tl[:, base:base + H * WP]
            return v.rearrange("p (h w) -> p h w", h=H, w=WP)[:, :, :W]

        def emit_gather(ci):
            slab = xpool.tile([CHP, SROW], u8, tag="slab")
            nc.gpsimd.indirect_dma_start(
                out=slab[:], out_offset=None, in_=xp[:, :],
                in_offset=bass.IndirectOffsetOnAxis(
                    ap=idx_t[:, ci:ci + 1], axis=1))
            return [crop3d(slab, c * CSTR) for c in range(C)]

        def emit_sums(ci, crops):
            ssum = tpool.tile([CHP, C], f32, tag="ssum")
            dumps = [None] * C
            for c in range(C):
                if PATH[(ci, c)] == "P1":
                    dump = dpool.tile([CHP, PX], u8, tag=f"dump{c}")
                    d3 = dump.rearrange("p (h w) -> p h w", h=H, w=W)
                    nc.scalar.activation(d3, crops[c], AF.Identity,
                                         accum_out=ssum[:, c:c + 1])
                    dumps[c] = dump
                else:
                    nc.vector.tensor_reduce(ssum[:, c:c + 1], crops[c],
                                            AX.XY, OP.add)
            return ssum, dumps

        def emit_tail(ci, crops, ssum, dumps):
            fT, dT, o4 = scal[ci]
            bT = tpool.tile([CHP, C], f32, tag="b")
            zu = opool.tile([CHP, OUTW], u8, tag="zu")
            rows = slice(CHP * ci, CHP * (ci + 1))
            for c in range(C):
                nc.vector.scalar_tensor_tensor(bT[:, c:c + 1], ssum[:, c:c + 1],
                                               o4[:], dT[:], OP.mult, OP.add)
                zslice = zu[:, c * PX:(c + 1) * PX]
                if PATH[(ci, c)] == "P1":
                    nc.vector.tensor_scalar(zslice, dumps[c][:, :],
                                            fT[:], bT[:, c:c + 1],
                                            OP.mult, OP.add)
                elif PATH[(ci, c)] == "P3":
                    z3 = zslice.rearrange("p (h w) -> p h w", h=H, w=W)
                    nc.vector.tensor_scalar(z3, crops[c], fT[:],
                                            bT[:, c:c + 1], OP.mult, OP.add)
                else:  # P2
                    z3 = zslice.rearrange("p (h w) -> p h w", h=H, w=W)
                    nc.scalar.activation(z3, crops[c], AF.Identity,
                                         bias=bT[:, c:c + 1], scale=fT[:])
                nc.sync.dma_start(out[rows, c * PX:(c + 1) * PX], zslice)

        # g0, s0, g1, t0, s1, t1: neither engine queue blocks on later data
        crops0 = emit_gather(0)
        s0 = emit_sums(0, crops0)
        crops1 = emit_gather(1)
        emit_tail(0, crops0, *s0)
        s1 = emit_sums(1, crops1)
        emit_tail(1, crops1, *s1)

    nc.compile()
    return nc


def _host_prep(x_uint8, offs_h, offs_w, jitter_b, jitter_c):
    """Shard + build per-core input maps (padding, dtype repack, and index
    arithmetic only - no image math)."""
    xpad = np.pad(np.asarray(x_uint8).astype(np.uint8),
                  ((0, 0), (0, 0), (PAD, PAD), (PAD, PAD)), mode="reflect")
    oh = np.asarray(offs_h).astype(np.int64).reshape(B)
    ow = np.asarray(offs_w).astype(np.int64).reshape(B)
    jb = np.asarray(jitter_b, dtype=np.float32).reshape(B)
    jc = np.asarray(jitter_c, dtype=np.float32).reshape(B)

    in_maps = []
    for k in range(NCORES):
        sl = slice(k * BS, (k + 1) * BS)
        start = (np.arange(BS, dtype=np.int64) * SROW
                 + oh[sl] * WP + ow[sl])                    # [BS] elem offsets
        idxm = start.reshape(NCHUNK, CHP).T.astype(np.int32).copy()
        jbrm = jb[sl].reshape(NCHUNK, CHP).T.copy()
        jcrm = jc[sl].reshape(NCHUNK, CHP).T.copy()
        xpf = np.zeros((1, BS * SROW + 1024), np.uint8)
        xpf[0, :BS * SROW] = xpad[sl].reshape(-1)
        in_maps.append({"xp": xpf, "idx": idxm, "jbr": jbrm, "jcr": jcrm})
    return in_maps


def kernel(x_uint8, offs_h, offs_w, jitter_b, jitter_c):
    global _prog, LAST_RESULT
    from concourse.bass_utils import run_bass_kernel_spmd

    if _prog is None:
        _prog = _build_program()

    in_maps = _host_prep(x_uint8, offs_h, offs_w, jitter_b, jitter_c)
    res = run_bass_kernel_spmd(_prog, in_maps, list(range(NCORES)), trace=TRACE)
    LAST_RESULT = res
    outs = [res.results[k]["out"].reshape(BS, C, H, W) for k in range(NCORES)]
    return np.concatenate(outs, axis=0).astype(np.int32)  # lossless: values in [0,255]
